# revision 9
# baseline (speedup 1.0000x reference)
"""GAT-style masked self-attention (B=4, N=4096, D=128) on 8 trn2 NeuronCores.

reference:
    scores = X @ X^T / sqrt(D)            [B, N, N]
    masked = where(adj > 0, scores, -1e12)
    attn   = softmax(masked, axis=2)
    out    = attn @ X                     [B, N, D]

Sharding: 8 cores <- (batch b, row-half h); each core handles 2048 rows
of one batch element against all 4096 keys.

Device algorithm (per core), orientation "S^T" (keys on partitions):
  - XT [D=128, 4096] in SBUF once (both matmul operands of the score MM).
  - For each 512-row block rb, for each key tile k (32 of them):
      psS = XT[:, k128].T @ XTrows[:, rb512]      # S^T tile [128 keys, 512 rows]
      p   = exp(scale * psS)                      # ACT, no max-subtraction
      ptm = p * adjT_tile                         # DVE/GpSimd 0/1 mask
  - AV with the denominator fused via an appended ones-column:
      psO[rc] (+)= ptm[:, k, rc128].T @ [X_k | 1] # [128 rows, 129] accum over k
      out = psO[:, :128] * (1 / psO[:, 128])      # row-wise normalize
  - softmax shift-invariance makes the max-subtraction unnecessary here:
    scores*scale are bounded (~|s|<16), exp stays well inside fp32 range.
  - row blocks are software-pipelined: while block rb runs scores/exp/mask,
    block rb-1 runs its AV matmuls (ptm double-buffered).

No collectives: every core produces its own 2048x128 output slice.
"""

import math
import sys

sys.path.insert(0, "/opt/trn_rl_repo")

import numpy as np

B, N, D = 4, 4096, 128
R = N // 2            # rows per core
NK = N // 128         # 32 key tiles
RB = 512              # rows per block
NRB = R // RB         # 4 row blocks
NCH = 16              # key tiles per adj DMA chunk
SCALE = 1.0 / math.sqrt(D)

# dtype / tuning config
CFG = dict(
    score_dt="float32r",  # score matmul operand dtype (float32 | float32r)
    p_dt="bfloat16",      # exp output / probs / xaug dtype (float32 | bfloat16)
    adj_dt="bfloat16",    # adjacency mask on the wire (float32 | bfloat16)
    ptm_bufs=2,           # 2 enables cross-row-block pipelining
    kg=4,                 # key tiles per PSUM score tile (= ACT batch)
    psum_s_bufs=1,
    adj_bufs=4,
    gp_mask=3,            # of every 8 mask-muls, this many go to GpSimd
)

_CACHE = {}


def _build_nc(cfg):
    from concourse import bacc
    import concourse.mybir as mybir
    from concourse.tile import TileContext

    dt = mybir.dt
    score_dt = getattr(dt, cfg["score_dt"])
    p_dt = getattr(dt, cfg["p_dt"])
    adj_dt = getattr(dt, cfg["adj_dt"])
    kg = cfg["kg"]
    gp_mask = cfg["gp_mask"]

    nc = bacc.Bacc(None, target_bir_lowering=False)

    xt_d = nc.dram_tensor("xt", [D, N], score_dt, kind="ExternalInput")
    xtr_d = nc.dram_tensor("xtr", [D, R], score_dt, kind="ExternalInput")
    xaug_d = nc.dram_tensor("xaug", [N, D + 1], p_dt, kind="ExternalInput")
    # adjT, host-packed as [rb, key_in_tile, key_tile, row_in_block]
    adj_d = nc.dram_tensor("adjt", [NRB, 128, NK, RB], adj_dt, kind="ExternalInput")
    o_d = nc.dram_tensor("o", [R, D], dt.float32, kind="ExternalOutput")

    with TileContext(nc) as tc:
        with (
            tc.tile_pool(name="singles", bufs=1) as singles,
            tc.tile_pool(name="ptm", bufs=cfg["ptm_bufs"]) as ptm_pool,
            tc.tile_pool(name="adj", bufs=cfg["adj_bufs"]) as adj_pool,
            tc.tile_pool(name="pe", bufs=2) as pe_pool,
            tc.tile_pool(name="outs", bufs=4) as out_pool,
            tc.tile_pool(name="small", bufs=4) as small_pool,
            tc.tile_pool(name="psS", bufs=cfg["psum_s_bufs"], space="PSUM") as psS_pool,
            tc.tile_pool(name="psO", bufs=4, space="PSUM") as psO_pool,
        ):
            # warm the exp table while the big init DMAs stream in
            warm = small_pool.tile([128, 1], mybir.dt.float32, tag="warm")
            nc.vector.memset(warm[:], 0.0)
            warm2 = small_pool.tile([128, 1], mybir.dt.float32, tag="warm")
            nc.scalar.activation(
                warm2[:], warm[:], mybir.ActivationFunctionType.Exp, scale=1.0
            )

            xt_sb = singles.tile([D, N], score_dt)
            for c in range(8):
                nc.gpsimd.dma_start(
                    out=xt_sb[:, c * 512:(c + 1) * 512],
                    in_=xt_d[:, c * 512:(c + 1) * 512],
                )
            xtr_sb = singles.tile([D, R], score_dt)
            for c in range(4):
                nc.gpsimd.dma_start(
                    out=xtr_sb[:, c * 512:(c + 1) * 512],
                    in_=xtr_d[:, c * 512:(c + 1) * 512],
                )
            xaug_sb = singles.tile([128, NK, D + 1], p_dt)
            nc.gpsimd.dma_start(
                out=xaug_sb[:],
                in_=xaug_d[:, :].rearrange("(t p) d -> p t d", p=128),
            )

            ptm_prev = None
            for rb in range(NRB + 1):
                ptm_cur = None
                psO = None
                if rb < NRB:
                    ptm_cur = ptm_pool.tile([128, NK, RB], p_dt, tag="ptm")
                    adj_sbs = []
                    for c in range(NK // NCH):
                        a = adj_pool.tile([128, NCH, RB], adj_dt, tag="adj",
                                          name=f"adj_{rb}_{c}")
                        nc.sync.dma_start(
                            out=a[:], in_=adj_d[rb, :, c * NCH:(c + 1) * NCH, :]
                        )
                        adj_sbs.append(a)
                if rb >= 1:
                    psO = [
                        psO_pool.tile(
                            [128, D + 1], mybir.dt.float32,
                            tag="psO", name=f"psO_{rb}_{rc}",
                        )
                        for rc in range(RB // 128)
                    ]

                for kgi in range(NK // kg):
                    # AV matmuls for the previous row block come first so PE
                    # has work while ACT drains this group's scores.
                    if rb >= 1:
                        for j in range(kg):
                            k = kgi * kg + j
                            for rc in range(RB // 128):
                                nc.tensor.matmul(
                                    psO[rc][:, :],
                                    lhsT=ptm_prev[:, k, rc * 128:(rc + 1) * 128],
                                    rhs=xaug_sb[:, k, :],
                                    start=(k == 0),
                                    stop=(k == NK - 1),
                                )
                    if rb < NRB:
                        ps = psS_pool.tile([128, kg, RB], mybir.dt.float32, tag="psS")
                        for j in range(kg):
                            k = kgi * kg + j
                            nc.tensor.matmul(
                                ps[:, j, :],
                                lhsT=xt_sb[:, k * 128:(k + 1) * 128],
                                rhs=xtr_sb[:, rb * RB:(rb + 1) * RB],
                                start=True,
                                stop=True,
                            )
                        pe_t = pe_pool.tile([128, kg, RB], p_dt, tag="pe")
                        nc.scalar.activation(
                            pe_t[:], ps[:],
                            mybir.ActivationFunctionType.Exp,
                            scale=SCALE,
                        )
                        for j in range(kg):
                            k = kgi * kg + j
                            a = adj_sbs[k // NCH]
                            eng = nc.gpsimd if (k % 8) < gp_mask else nc.vector
                            eng.tensor_mul(
                                ptm_cur[:, k, :], pe_t[:, j, :], a[:, k % NCH, :]
                            )
                if rb >= 1:
                    for rc in range(RB // 128):
                        recip = small_pool.tile([128, 1], mybir.dt.float32, tag="recip")
                        nc.vector.reciprocal(recip[:], psO[rc][:, D:D + 1])
                        o_sb = out_pool.tile([128, D], mybir.dt.float32, tag="o")
                        nc.vector.tensor_scalar_mul(o_sb[:], psO[rc][:, 0:D], recip[:])
                        r0 = (rb - 1) * RB + rc * 128
                        nc.sync.dma_start(out=o_d[r0:r0 + 128, :], in_=o_sb[:])
                ptm_prev = ptm_cur
    nc.finalize()
    return nc


def _get_nc():
    key = tuple(sorted(CFG.items()))
    if key not in _CACHE:
        _CACHE[key] = _build_nc(CFG)
    return _CACHE[key]


def _np_dt(name):
    import ml_dtypes

    return {
        "float32": np.float32,
        "float32r": np.float32,
        "bfloat16": ml_dtypes.bfloat16,
    }[name]


def make_in_maps(input, adj):
    """Host-side shard/layout prep: one input map per core."""
    input = np.asarray(input, dtype=np.float32)
    adj = np.asarray(adj)
    score_np = _np_dt(CFG["score_dt"])
    p_np = _np_dt(CFG["p_dt"])
    adj_np = _np_dt(CFG["adj_dt"])

    in_maps = []
    for core in range(8):
        b, h = core // 2, core % 2
        xb = input[b]                                    # [N, D]
        xt = np.ascontiguousarray(xb.T).astype(score_np, copy=False)
        xtr = np.ascontiguousarray(xb.T[:, h * R:(h + 1) * R]).astype(
            score_np, copy=False
        )
        xaug = np.concatenate([xb, np.ones((N, 1), np.float32)], axis=1)
        xaug = np.ascontiguousarray(xaug).astype(p_np)
        s = adj[b][h * R:(h + 1) * R, :]                 # [R rows, N cols]
        # adjt[rb, p, k, r] = s[rb*512 + r, k*128 + p]
        adjt = np.ascontiguousarray(
            s.reshape(NRB, RB, NK, 128).transpose(0, 3, 2, 1)
        ).astype(adj_np)
        in_maps.append({"xt": xt, "xtr": xtr, "xaug": xaug, "adjt": adjt})
    return in_maps


def run_device(in_maps, trace=False, trace_cores=None):
    import concourse.bass_utils as bass_utils

    if trace:
        bass_utils.upload_artifacts = lambda tmpdir: ""  # no bucket in sandbox
    nc = _get_nc()
    return bass_utils.run_bass_kernel_spmd(
        nc, in_maps, list(range(8)), trace=trace, trace_cores=trace_cores
    )


def kernel(input, adj):
    res = run_device(make_in_maps(input, adj))
    out = np.empty((B, N, D), dtype=np.float32)
    for core in range(8):
        b, h = core // 2, core % 2
        out[b, h * R:(h + 1) * R, :] = res.results[core]["o"]
    return out


# revision 10
# speedup vs baseline: 1.1575x; 1.1575x over previous
"""GAT-style masked self-attention (B=4, N=4096, D=128) on 8 trn2 NeuronCores.

reference:
    scores = X @ X^T / sqrt(D)            [B, N, N]
    masked = where(adj > 0, scores, -1e12)
    attn   = softmax(masked, axis=2)
    out    = attn @ X                     [B, N, D]

Sharding: 8 cores <- (batch b, row-half h); each core handles 2048 rows
of one batch element against all 4096 keys.

Device algorithm (per core), orientation "S^T" (keys on partitions):
  - XT [D=128, 4096] in SBUF once (both matmul operands of the score MM).
  - For each 512-row block rb, for each key tile k (32 of them):
      psS = XT[:, k128].T @ XTrows[:, rb512]      # S^T tile [128 keys, 512 rows]
      p   = exp(scale * psS)                      # ACT, no max-subtraction
      ptm = p * adjT_tile                         # DVE/GpSimd 0/1 mask
  - AV with the denominator fused via an appended ones-column:
      psO[rc] (+)= ptm[:, k, rc128].T @ [X_k | 1] # [128 rows, 129] accum over k
      out = psO[:, :128] * (1 / psO[:, 128])      # row-wise normalize
  - softmax shift-invariance makes the max-subtraction unnecessary here:
    scores*scale are bounded (~|s|<16), exp stays well inside fp32 range.
  - row blocks are software-pipelined: while block rb runs scores/exp/mask,
    block rb-1 runs its AV matmuls (ptm double-buffered).

No collectives: every core produces its own 2048x128 output slice.
"""

import math
import sys

sys.path.insert(0, "/opt/trn_rl_repo")

import numpy as np

B, N, D = 4, 4096, 128
R = N // 2            # rows per core
NK = N // 128         # 32 key tiles
RB = 512              # rows per block
NRB = R // RB         # 4 row blocks
NCH = 16              # key tiles per adj DMA chunk
SCALE = 1.0 / math.sqrt(D)

# dtype / tuning config
CFG = dict(
    score_dt="float32r",  # score matmul operand dtype (float32 | float32r)
    p_dt="bfloat16",      # exp output / probs / xaug dtype (float32 | bfloat16)
    adj_dt="bfloat16",    # adjacency mask on the wire (float32 | bfloat16)
    ptm_bufs=2,           # 2 enables cross-row-block pipelining
    kg=2,                 # key tiles per PSUM score tile (= ACT batch)
    psum_s_bufs=2,
    adj_bufs=4,
    gp_mask=3,            # of every 8 mask groups, this many go to GpSimd
)

_CACHE = {}


def _build_nc(cfg):
    from concourse import bacc
    import concourse.mybir as mybir
    from concourse.tile import TileContext

    dt = mybir.dt
    score_dt = getattr(dt, cfg["score_dt"])
    p_dt = getattr(dt, cfg["p_dt"])
    adj_dt = getattr(dt, cfg["adj_dt"])
    kg = cfg["kg"]
    gp_mask = cfg["gp_mask"]

    nc = bacc.Bacc(None, target_bir_lowering=False)

    xt_d = nc.dram_tensor("xt", [D, N], score_dt, kind="ExternalInput")
    xtr_d = nc.dram_tensor("xtr", [D, R], score_dt, kind="ExternalInput")
    xaug_d = nc.dram_tensor("xaug", [N, D + 1], p_dt, kind="ExternalInput")
    # adjT, host-packed as [rb, key_in_tile, key_tile, row_in_block]
    adj_d = nc.dram_tensor("adjt", [NRB, 128, NK, RB], adj_dt, kind="ExternalInput")
    o_d = nc.dram_tensor("o", [R, D], dt.float32, kind="ExternalOutput")

    with TileContext(nc) as tc:
        with (
            tc.tile_pool(name="singles", bufs=1) as singles,
            tc.tile_pool(name="ptm", bufs=cfg["ptm_bufs"]) as ptm_pool,
            tc.tile_pool(name="adj", bufs=cfg["adj_bufs"]) as adj_pool,
            tc.tile_pool(name="pe", bufs=2) as pe_pool,
            tc.tile_pool(name="outs", bufs=4) as out_pool,
            tc.tile_pool(name="small", bufs=4) as small_pool,
            tc.tile_pool(name="psS", bufs=cfg["psum_s_bufs"], space="PSUM") as psS_pool,
            tc.tile_pool(name="psO", bufs=4, space="PSUM") as psO_pool,
        ):
            # warm the exp table while the big init DMAs stream in
            warm = small_pool.tile([128, 1], mybir.dt.float32, tag="warm")
            nc.vector.memset(warm[:], 0.0)
            warm2 = small_pool.tile([128, 1], mybir.dt.float32, tag="warm")
            nc.scalar.activation(
                warm2[:], warm[:], mybir.ActivationFunctionType.Exp, scale=1.0
            )

            xt_sb = singles.tile([D, N], score_dt)
            for c in range(8):
                nc.gpsimd.dma_start(
                    out=xt_sb[:, c * 512:(c + 1) * 512],
                    in_=xt_d[:, c * 512:(c + 1) * 512],
                )
            xtr_sb = singles.tile([D, R], score_dt)
            for c in range(4):
                nc.gpsimd.dma_start(
                    out=xtr_sb[:, c * 512:(c + 1) * 512],
                    in_=xtr_d[:, c * 512:(c + 1) * 512],
                )
            xaug_sb = singles.tile([128, NK, D + 1], p_dt)
            nc.gpsimd.dma_start(
                out=xaug_sb[:],
                in_=xaug_d[:, :].rearrange("(t p) d -> p t d", p=128),
            )

            ptm_prev = None
            for rb in range(NRB + 1):
                ptm_cur = None
                psO = None
                if rb < NRB:
                    ptm_cur = ptm_pool.tile([128, NK, RB], p_dt, tag="ptm")
                    adj_sbs = []
                    for c in range(NK // NCH):
                        a = adj_pool.tile([128, NCH, RB], adj_dt, tag="adj",
                                          name=f"adj_{rb}_{c}")
                        nc.sync.dma_start(
                            out=a[:], in_=adj_d[rb, :, c * NCH:(c + 1) * NCH, :]
                        )
                        adj_sbs.append(a)
                if rb >= 1:
                    psO = [
                        psO_pool.tile(
                            [128, D + 1], mybir.dt.float32,
                            tag="psO", name=f"psO_{rb}_{rc}",
                        )
                        for rc in range(RB // 128)
                    ]

                for kgi in range(NK // kg):
                    # AV matmuls for the previous row block come first so PE
                    # has work while ACT drains this group's scores.
                    if rb >= 1:
                        for j in range(kg):
                            k = kgi * kg + j
                            for rc in range(RB // 128):
                                nc.tensor.matmul(
                                    psO[rc][:, :],
                                    lhsT=ptm_prev[:, k, rc * 128:(rc + 1) * 128],
                                    rhs=xaug_sb[:, k, :],
                                    start=(k == 0),
                                    stop=(k == NK - 1),
                                )
                    if rb < NRB:
                        ps = psS_pool.tile([128, kg, RB], mybir.dt.float32, tag="psS")
                        for j in range(kg):
                            k = kgi * kg + j
                            nc.tensor.matmul(
                                ps[:, j, :],
                                lhsT=xt_sb[:, k * 128:(k + 1) * 128],
                                rhs=xtr_sb[:, rb * RB:(rb + 1) * RB],
                                start=True,
                                stop=True,
                            )
                        pe_t = pe_pool.tile([128, kg, RB], p_dt, tag="pe")
                        nc.scalar.activation(
                            pe_t[:], ps[:],
                            mybir.ActivationFunctionType.Exp,
                            scale=SCALE,
                        )
                        k0 = kgi * kg
                        a = adj_sbs[k0 // NCH]
                        eng = nc.gpsimd if (kgi % 8) < gp_mask else nc.vector
                        eng.tensor_mul(
                            ptm_cur[:, k0:k0 + kg, :],
                            pe_t[:, :, :],
                            a[:, k0 % NCH:k0 % NCH + kg, :],
                        )
                if rb >= 1:
                    for rc in range(RB // 128):
                        recip = small_pool.tile([128, 1], mybir.dt.float32, tag="recip")
                        nc.vector.reciprocal(recip[:], psO[rc][:, D:D + 1])
                        o_sb = out_pool.tile([128, D], mybir.dt.float32, tag="o")
                        nc.vector.tensor_scalar_mul(o_sb[:], psO[rc][:, 0:D], recip[:])
                        r0 = (rb - 1) * RB + rc * 128
                        nc.sync.dma_start(out=o_d[r0:r0 + 128, :], in_=o_sb[:])
                ptm_prev = ptm_cur
    nc.finalize()
    return nc


def _get_nc():
    key = tuple(sorted(CFG.items()))
    if key not in _CACHE:
        _CACHE[key] = _build_nc(CFG)
    return _CACHE[key]


def _np_dt(name):
    import ml_dtypes

    return {
        "float32": np.float32,
        "float32r": np.float32,
        "bfloat16": ml_dtypes.bfloat16,
    }[name]


def make_in_maps(input, adj):
    """Host-side shard/layout prep: one input map per core."""
    input = np.asarray(input, dtype=np.float32)
    adj = np.asarray(adj)
    score_np = _np_dt(CFG["score_dt"])
    p_np = _np_dt(CFG["p_dt"])
    adj_np = _np_dt(CFG["adj_dt"])

    in_maps = []
    for core in range(8):
        b, h = core // 2, core % 2
        xb = input[b]                                    # [N, D]
        xt = np.ascontiguousarray(xb.T).astype(score_np, copy=False)
        xtr = np.ascontiguousarray(xb.T[:, h * R:(h + 1) * R]).astype(
            score_np, copy=False
        )
        xaug = np.concatenate([xb, np.ones((N, 1), np.float32)], axis=1)
        xaug = np.ascontiguousarray(xaug).astype(p_np)
        s = adj[b][h * R:(h + 1) * R, :]                 # [R rows, N cols]
        # adjt[rb, p, k, r] = s[rb*512 + r, k*128 + p]
        adjt = np.ascontiguousarray(
            s.reshape(NRB, RB, NK, 128).transpose(0, 3, 2, 1)
        ).astype(adj_np)
        in_maps.append({"xt": xt, "xtr": xtr, "xaug": xaug, "adjt": adjt})
    return in_maps


def run_device(in_maps, trace=False, trace_cores=None):
    import concourse.bass_utils as bass_utils

    if trace:
        bass_utils.upload_artifacts = lambda tmpdir: ""  # no bucket in sandbox
    nc = _get_nc()
    return bass_utils.run_bass_kernel_spmd(
        nc, in_maps, list(range(8)), trace=trace, trace_cores=trace_cores
    )


def kernel(input, adj):
    res = run_device(make_in_maps(input, adj))
    out = np.empty((B, N, D), dtype=np.float32)
    for core in range(8):
        b, h = core // 2, core % 2
        out[b, h * R:(h + 1) * R, :] = res.results[core]["o"]
    return out


# revision 11
# speedup vs baseline: 1.3524x; 1.1683x over previous
"""GAT-style masked self-attention (B=4, N=4096, D=128) on 8 trn2 NeuronCores.

reference:
    scores = X @ X^T / sqrt(D)            [B, N, N]
    masked = where(adj > 0, scores, -1e12)
    attn   = softmax(masked, axis=2)
    out    = attn @ X                     [B, N, D]

Sharding: 8 cores <- (batch b, row-half h); each core handles 2048 rows
of one batch element against all 4096 keys. No collectives: every core
produces its own 2048x128 output slice.

Device algorithm (per core), orientation "S^T" (keys on partitions):
  - score matmul (float32r, full PE rate): psS = XT[:,k128].T @ XTrows[:,rb512]
  - DVE evicts PSUM with the mask folded in:  sm = psS + adjM
    (adjM is 0 where adj=1, -1e30 where adj=0, so exp gives exact 0)
  - ACT exp in big SBUF-sourced ops:          ptm = exp(scale * sm)   (bf16)
  - AV matmul with the denominator fused via an appended ones-column:
      psO[rc] (+)= ptm[:, k, rc128].T @ [X_k | 1]   accumulated over k
      out = psO[:, :128] * (1 / psO[:, 128])        row-wise normalize
  - softmax shift-invariance makes max-subtraction unnecessary here:
    scores*scale are bounded (~|s|<16), exp stays well inside fp32 range.
  - row blocks are software-pipelined: block rb runs scores/mask/exp while
    block rb-1 runs its AV matmuls (ptm double-buffered); AV matmuls are
    emitted first within each group so PE covers the DVE/ACT drain.
"""

import math
import sys

sys.path.insert(0, "/opt/trn_rl_repo")

import numpy as np

B, N, D = 4, 4096, 128
R = N // 2            # rows per core
NK = N // 128         # 32 key tiles
RB = 512              # rows per block
NRB = R // RB         # 4 row blocks
NCH = 16              # key tiles per adjM DMA chunk
SG = 8                # key tiles per exp ACT instruction
SCALE = 1.0 / math.sqrt(D)
MASK_BIG = -1e30

CFG = dict(
    score_dt="float32r",  # score matmul operand dtype (float32 | float32r)
    p_dt="bfloat16",      # probs / xaug dtype
    adj_dt="bfloat16",    # additive mask on the wire {0, -1e30}
    ptm_bufs=2,
    kg=2,                 # key tiles per PSUM score tile (= DVE evict batch)
    psum_s_bufs=2,
    adj_bufs=4,
    sm_bufs=2,
)

_CACHE = {}


def _build_nc(cfg):
    from concourse import bacc
    import concourse.mybir as mybir
    from concourse.tile import TileContext

    dt = mybir.dt
    score_dt = getattr(dt, cfg["score_dt"])
    p_dt = getattr(dt, cfg["p_dt"])
    adj_dt = getattr(dt, cfg["adj_dt"])
    kg = cfg["kg"]

    nc = bacc.Bacc(None, target_bir_lowering=False)

    xt_d = nc.dram_tensor("xt", [D, N], score_dt, kind="ExternalInput")
    xtr_d = nc.dram_tensor("xtr", [D, R], score_dt, kind="ExternalInput")
    xaug_d = nc.dram_tensor("xaug", [N, D + 1], p_dt, kind="ExternalInput")
    # additive mask, host-packed as [rb, key_in_tile, key_tile, row_in_block]
    adj_d = nc.dram_tensor("adjt", [NRB, 128, NK, RB], adj_dt, kind="ExternalInput")
    o_d = nc.dram_tensor("o", [R, D], dt.float32, kind="ExternalOutput")

    with TileContext(nc) as tc:
        with (
            tc.tile_pool(name="singles", bufs=1) as singles,
            tc.tile_pool(name="ptm", bufs=cfg["ptm_bufs"]) as ptm_pool,
            tc.tile_pool(name="adj", bufs=cfg["adj_bufs"]) as adj_pool,
            tc.tile_pool(name="sm", bufs=cfg["sm_bufs"]) as sm_pool,
            tc.tile_pool(name="outs", bufs=4) as out_pool,
            tc.tile_pool(name="small", bufs=4) as small_pool,
            tc.tile_pool(name="psS", bufs=cfg["psum_s_bufs"], space="PSUM") as psS_pool,
            tc.tile_pool(name="psO", bufs=4, space="PSUM") as psO_pool,
        ):
            # warm the exp table while the init DMAs stream in
            warm = small_pool.tile([128, 1], mybir.dt.float32, tag="warm")
            nc.vector.memset(warm[:], 0.0)
            warm2 = small_pool.tile([128, 1], mybir.dt.float32, tag="warm")
            nc.scalar.activation(
                warm2[:], warm[:], mybir.ActivationFunctionType.Exp, scale=1.0
            )

            # init DMAs, ordered by first consumption; xt/xtr on the sync
            # HWDGE ring ahead of adj, xaug on the idle SWDGE ring.
            xt_sb = singles.tile([D, N], score_dt)
            xtr_sb = singles.tile([D, R], score_dt)
            nc.sync.dma_start(out=xt_sb[:, 0:512], in_=xt_d[:, 0:512])
            nc.sync.dma_start(out=xtr_sb[:, 0:512], in_=xtr_d[:, 0:512])
            for c in range(1, 8):
                nc.sync.dma_start(
                    out=xt_sb[:, c * 512:(c + 1) * 512],
                    in_=xt_d[:, c * 512:(c + 1) * 512],
                )
            for c in range(1, 4):
                nc.gpsimd.dma_start(
                    out=xtr_sb[:, c * 512:(c + 1) * 512],
                    in_=xtr_d[:, c * 512:(c + 1) * 512],
                )
            xaug_sb = singles.tile([128, NK, D + 1], p_dt)
            nc.gpsimd.dma_start(
                out=xaug_sb[:],
                in_=xaug_d[:, :].rearrange("(t p) d -> p t d", p=128),
            )

            ptm_prev = None
            for rb in range(NRB + 1):
                ptm_cur = None
                psO = None
                if rb < NRB:
                    ptm_cur = ptm_pool.tile([128, NK, RB], p_dt, tag="ptm")
                    adj_sbs = []
                    for c in range(NK // NCH):
                        a = adj_pool.tile([128, NCH, RB], adj_dt, tag="adj",
                                          name=f"adj_{rb}_{c}")
                        nc.sync.dma_start(
                            out=a[:], in_=adj_d[rb, :, c * NCH:(c + 1) * NCH, :]
                        )
                        adj_sbs.append(a)
                if rb >= 1:
                    psO = [
                        psO_pool.tile(
                            [128, D + 1], mybir.dt.float32,
                            tag="psO", name=f"psO_{rb}_{rc}",
                        )
                        for rc in range(RB // 128)
                    ]

                for sg in range(NK // SG):
                    smt = None
                    if rb < NRB:
                        smt = sm_pool.tile([128, SG, RB], mybir.dt.float32, tag="sm",
                                           name=f"sm_{rb}_{sg}")
                    for kgi in range(SG // kg):
                        # AV matmuls for the previous row block first: PE has
                        # work while DVE/ACT drain this group's scores.
                        if rb >= 1:
                            for j in range(kg):
                                k = sg * SG + kgi * kg + j
                                for rc in range(RB // 128):
                                    nc.tensor.matmul(
                                        psO[rc][:, :],
                                        lhsT=ptm_prev[:, k, rc * 128:(rc + 1) * 128],
                                        rhs=xaug_sb[:, k, :],
                                        start=(k == 0),
                                        stop=(k == NK - 1),
                                    )
                        if rb < NRB:
                            ps = psS_pool.tile([128, kg, RB], mybir.dt.float32,
                                               tag="psS")
                            for j in range(kg):
                                k = sg * SG + kgi * kg + j
                                nc.tensor.matmul(
                                    ps[:, j, :],
                                    lhsT=xt_sb[:, k * 128:(k + 1) * 128],
                                    rhs=xtr_sb[:, rb * RB:(rb + 1) * RB],
                                    start=True,
                                    stop=True,
                                )
                            # evict PSUM with additive mask folded in
                            k0 = sg * SG + kgi * kg
                            a = adj_sbs[k0 // NCH]
                            nc.vector.tensor_add(
                                smt[:, kgi * kg:(kgi + 1) * kg, :],
                                ps[:, :, :],
                                a[:, k0 % NCH:k0 % NCH + kg, :],
                            )
                    if rb < NRB:
                        nc.scalar.activation(
                            ptm_cur[:, sg * SG:(sg + 1) * SG, :],
                            smt[:, :, :],
                            mybir.ActivationFunctionType.Exp,
                            scale=SCALE,
                        )
                if rb >= 1:
                    for rc in range(RB // 128):
                        recip = small_pool.tile([128, 1], mybir.dt.float32,
                                                tag="recip")
                        nc.vector.reciprocal(recip[:], psO[rc][:, D:D + 1])
                        o_sb = out_pool.tile([128, D], mybir.dt.float32, tag="o")
                        nc.vector.tensor_scalar_mul(o_sb[:], psO[rc][:, 0:D], recip[:])
                        r0 = (rb - 1) * RB + rc * 128
                        nc.sync.dma_start(out=o_d[r0:r0 + 128, :], in_=o_sb[:])
                ptm_prev = ptm_cur
    nc.finalize()
    return nc


def _get_nc():
    key = tuple(sorted(CFG.items()))
    if key not in _CACHE:
        _CACHE[key] = _build_nc(CFG)
    return _CACHE[key]


def _np_dt(name):
    import ml_dtypes

    return {
        "float32": np.float32,
        "float32r": np.float32,
        "bfloat16": ml_dtypes.bfloat16,
    }[name]


def make_in_maps(input, adj):
    """Host-side shard/layout prep: one input map per core."""
    input = np.asarray(input, dtype=np.float32)
    adj = np.asarray(adj)
    score_np = _np_dt(CFG["score_dt"])
    p_np = _np_dt(CFG["p_dt"])
    adj_np = _np_dt(CFG["adj_dt"])

    in_maps = []
    for core in range(8):
        b, h = core // 2, core % 2
        xb = input[b]                                    # [N, D]
        xt = np.ascontiguousarray(xb.T).astype(score_np, copy=False)
        xtr = np.ascontiguousarray(xb.T[:, h * R:(h + 1) * R]).astype(
            score_np, copy=False
        )
        xaug = np.concatenate([xb, np.ones((N, 1), np.float32)], axis=1)
        xaug = np.ascontiguousarray(xaug).astype(p_np)
        s = adj[b][h * R:(h + 1) * R, :]                 # [R rows, N cols]
        # additive mask: 0 where edge, -1e30 where no edge
        # adjt[rb, p, k, r] = mask(s[rb*512 + r, k*128 + p])
        adjm = np.where(s > 0, np.float32(0.0), np.float32(MASK_BIG))
        adjt = np.ascontiguousarray(
            adjm.reshape(NRB, RB, NK, 128).transpose(0, 3, 2, 1)
        ).astype(adj_np)
        in_maps.append({"xt": xt, "xtr": xtr, "xaug": xaug, "adjt": adjt})
    return in_maps


def run_device(in_maps, trace=False, trace_cores=None):
    import concourse.bass_utils as bass_utils

    if trace:
        bass_utils.upload_artifacts = lambda tmpdir: ""  # no bucket in sandbox
    nc = _get_nc()
    return bass_utils.run_bass_kernel_spmd(
        nc, in_maps, list(range(8)), trace=trace, trace_cores=trace_cores
    )


def kernel(input, adj):
    res = run_device(make_in_maps(input, adj))
    out = np.empty((B, N, D), dtype=np.float32)
    for core in range(8):
        b, h = core // 2, core % 2
        out[b, h * R:(h + 1) * R, :] = res.results[core]["o"]
    return out
